# revision 2
# baseline (speedup 1.0000x reference)
"""Trainium2 Bass kernel for nn_MultiHeadAttention_61546881352366.

The reference module's observable output is NOT attention: the attention
result is dead code in the original torch module.  The output is

    out = fc0(concat_h(v @ Wv_h^T)) = (v @ Wcat^T) @ W0^T + b0

with Wcat = Wv.reshape(H*D, C).  Two chained linear maps fuse into one:

    out = v @ (W0 @ Wcat)^T + b0 = v @ WcT + b0,   WcT = (W0 @ Wcat)^T

so the device work is a single [B*T, C] @ [C, C] matmul.  k and q are
unused.  The bias add and the final upcast to fp32 happen on the host.

Sharding: data-parallel over batch (B == 8 == n_cores); each core computes
one batch element's [2048, 1024] @ [1024, 1024] product in bf16 (fp32 PSUM
accumulate; rel err ~2e-3 incl. the bf16 output rounding).  Weights are
replicated (2 MiB/core).

Device kernel (per core):
  - inputs laid out on the host so every DMA moves 4 KiB contiguous
    per-partition lines:
      vp [128, 16, 8, 128] bf16   vp[p,m,k,t] = v[m*128+t, k*128+p]
      wp [128, 8, 1024]   bf16    wp[p,k,j]   = WcT[k*128+p, j]
  - 256 matmuls of [128x128] @ [128x512] bf16 at the warm PE rate
    (~216 ns each); k-outer fill phase over the first 4 row tiles so the
    PE never starves while w streams in; warmup matmuls ramp the HAM
    clock gate during the DMA fill so real matmuls run at 2.4 GHz
  - w DMAs trigger on the scalar engine, v DMAs on gpsimd (parallel
    queues), output DMAs on sync
  - per row tile: one [128,1024] PSUM tile (both column halves), one
    vector copy fp32->bf16 into SBUF, one 256 KiB output DMA; the last
    row tile drains in halves so only half a drain trails the final
    matmul
"""

import numpy as np

import concourse.bacc as bacc
import concourse.mybir as mybir
from concourse.tile import TileContext
from concourse.bass_utils import run_bass_kernel_spmd

B, T, C = 8, 2048, 1024
H, D = 16, 64
P = 128
KT = C // P   # 8 contraction tiles
MT = T // P   # 16 row tiles per core
NF = 512      # matmul moving free dim (= one PSUM bank of fp32)
NJ = C // NF  # 2 output column halves

_FP32 = mybir.dt.float32
_BF16 = mybir.dt.bfloat16

G = 4         # row tiles covered by the k-outer fill phase
N_WARMUP = 8  # dummy matmuls to ramp the PE clock during the DMA fill
V_GROUPS = [(0, 2), (2, 4), (4, 8), (8, 12), (12, 16)]


def _build():
    nc = bacc.Bacc()
    vp = nc.dram_tensor("vp", [P, MT, KT, P], _BF16, kind="ExternalInput")
    wp = nc.dram_tensor("wp", [P, KT, C], _BF16, kind="ExternalInput")
    out = nc.dram_tensor("out", [T, C], _BF16, kind="ExternalOutput")

    with TileContext(nc) as tc:
        with (
            tc.tile_pool(name="wpool", bufs=1) as wpool,
            tc.tile_pool(name="vpool", bufs=1) as vpool,
            tc.tile_pool(name="spool", bufs=1) as spool,
            tc.tile_pool(name="opool", bufs=4) as opool,
            tc.tile_pool(name="pspool", bufs=4, space="PSUM") as pspool,
        ):
            # PE warmup: dependency-free matmuls on a memset tile so the HAM
            # clock gate ramps to 2.4 GHz while the first DMAs are in flight.
            scratch = spool.tile([P, NF], _BF16, name="scratch", tag="scratch")
            nc.vector.memset(scratch, 0.0)
            ps_w = pspool.tile([P, NF], _FP32, name="ps_w", tag="ps")
            for _ in range(N_WARMUP):
                nc.tensor.matmul(
                    ps_w, lhsT=scratch[:, :P], rhs=scratch, start=True, stop=True
                )

            # Input DMAs.  w chunks (2 k-tiles x full C) on the scalar
            # engine's queue; v strip groups on gpsimd's queue in parallel.
            w_sb = []
            for c in range(KT // 2):
                w_c = wpool.tile([P, 2, C], _BF16, name=f"w_{c}", tag=f"w_{c}")
                nc.scalar.dma_start(out=w_c, in_=wp[:, 2 * c : 2 * c + 2, :])
                w_sb.append(w_c)
            v_sb = [None] * MT
            for lo, hi in V_GROUPS:
                vt = vpool.tile(
                    [P, hi - lo, KT, P], _BF16, name=f"v_{lo}", tag=f"v_{lo}"
                )
                nc.gpsimd.dma_start(out=vt, in_=vp[:, lo:hi, :, :])
                for m in range(lo, hi):
                    v_sb[m] = (vt, m - lo)

            def mm(ps_m, m, k, j):
                vt, s = v_sb[m]
                nc.tensor.matmul(
                    ps_m[:, j * NF : (j + 1) * NF],
                    lhsT=vt[:, s, k, :],
                    rhs=w_sb[k // 2][:, k % 2, j * NF : (j + 1) * NF],
                    start=(k == 0),
                    stop=(k == KT - 1),
                )

            ps = {}

            def drain(m, ob_m, j=None):
                sl = slice(None) if j is None else slice(j * NF, (j + 1) * NF)
                nc.vector.tensor_copy(ob_m[:, sl], ps[m][:, sl])
                nc.sync.dma_start(out=out[m * P : (m + 1) * P, sl], in_=ob_m[:, sl])

            # Fill phase (m0-3): k-outer so each arriving w chunk immediately
            # feeds 16 matmuls -- the PE never idles while w streams in.
            for m in range(G):
                ps[m] = pspool.tile([P, C], _FP32, name=f"ps_{m}", tag="ps")
            for c in range(KT // 2):
                for m in range(G):
                    for kk in range(2):
                        for j in range(NJ):
                            mm(ps[m], m, 2 * c + kk, j)
                    if c == KT // 2 - 1:
                        ob_m = opool.tile([P, C], _BF16, name=f"ob_{m}", tag="ob")
                        drain(m, ob_m)

            # Steady phase (m4-15): m-major, one drain per row tile.  The
            # last tile runs j-outer and drains in halves so only half a
            # drain trails the final matmul.
            for m in range(G, MT):
                ps[m] = pspool.tile([P, C], _FP32, name=f"ps_{m}", tag="ps")
                ob_m = opool.tile([P, C], _BF16, name=f"ob_{m}", tag="ob")
                if m < MT - 1:
                    for k in range(KT):
                        for j in range(NJ):
                            mm(ps[m], m, k, j)
                    drain(m, ob_m)
                else:
                    for j in range(NJ):
                        for k in range(KT):
                            mm(ps[m], m, k, j)
                        drain(m, ob_m, j=j)
    nc.compile()
    return nc


_nc_cache = None


def _get_nc():
    global _nc_cache
    if _nc_cache is None:
        _nc_cache = _build()
    return _nc_cache


def prepare_inputs(inputs):
    """Host-side prep shared by kernel() and the timing harness.

    Returns (in_maps, b0): per-core device inputs and the bias to add on
    the host after the gather.
    """
    import ml_dtypes

    v = np.ascontiguousarray(np.asarray(inputs["v"], dtype=np.float32))
    Wv = np.asarray(inputs["Wv"], dtype=np.float32)
    W0 = np.asarray(inputs["W0"], dtype=np.float32)
    b0 = np.asarray(inputs["b0"], dtype=np.float32)

    # Fuse the two linear layers on the host: WcT = (W0 @ Wcat)^T  [C_in, C_out]
    Wc = W0 @ Wv.reshape(H * D, C)
    # wp[p, k, j] = WcT[k*128+p, j]
    wp = np.ascontiguousarray(
        Wc.T.reshape(KT, P, C).transpose(1, 0, 2).astype(ml_dtypes.bfloat16)
    )
    # vp[b][p, m, k, t] = v[b, m*128+t, k*128+p]
    vp = np.ascontiguousarray(
        v.reshape(B, MT, P, KT, P).transpose(0, 4, 1, 3, 2).astype(ml_dtypes.bfloat16)
    )
    return [{"vp": vp[i], "wp": wp} for i in range(B)], b0


def kernel(**inputs):
    in_maps, b0 = prepare_inputs(inputs)
    nc = _get_nc()
    res = run_bass_kernel_spmd(nc, in_maps, core_ids=list(range(B)))
    out = np.stack([res.results[i]["out"] for i in range(B)], axis=0)
    return out.astype(np.float32) + b0


# revision 7
# speedup vs baseline: 1.0286x; 1.0286x over previous
"""Trainium2 Bass kernel for nn_MultiHeadAttention_61546881352366.

The reference module's observable output is NOT attention: the attention
result is dead code in the original torch module.  The output is

    out = fc0(concat_h(v @ Wv_h^T)) = (v @ Wcat^T) @ W0^T + b0

with Wcat = Wv.reshape(H*D, C).  Two chained linear maps fuse into one:

    out = v @ (W0 @ Wcat)^T + b0 = v @ WcT + b0,   WcT = (W0 @ Wcat)^T

so the device work is a single [B*T, C] @ [C, C] matmul.  k and q are
unused.  The bias add and the final upcast to fp32 happen on the host.

Sharding: data-parallel over batch (B == 8 == n_cores); each core computes
one batch element's [2048, 1024] @ [1024, 1024] product in bf16 (fp32 PSUM
accumulate; rel err ~2e-3 incl. the bf16 output rounding).  Weights are
replicated (2 MiB/core).

Device kernel (per core):
  - inputs laid out on the host so every DMA moves 4 KiB contiguous
    per-partition lines:
      vp [128, 16, 8, 128] bf16   vp[p,m,k,t] = v[m*128+t, k*128+p]
      wp [128, 8, 1024]   bf16    wp[p,k,j]   = WcT[k*128+p, j]
  - 256 matmuls of [128x128] @ [128x512] bf16 at the warm PE rate
    (~216 ns each); k-outer fill phase over the first 4 row tiles so the
    PE never starves while w streams in; warmup matmuls ramp the HAM
    clock gate during the DMA fill so real matmuls run at 2.4 GHz
  - w DMAs trigger on the scalar engine, v DMAs on gpsimd (parallel
    queues), output DMAs on sync
  - per row tile: one [128,1024] PSUM tile (both column halves), one
    vector copy fp32->bf16 into SBUF, one 256 KiB output DMA; the last
    row tile drains in halves so only half a drain trails the final
    matmul
"""

import numpy as np

import concourse.bacc as bacc
import concourse.mybir as mybir
from concourse.tile import TileContext
from concourse.bass_utils import run_bass_kernel_spmd

B, T, C = 8, 2048, 1024
H, D = 16, 64
P = 128
KT = C // P   # 8 contraction tiles
MT = T // P   # 16 row tiles per core
NF = 512      # matmul moving free dim (= one PSUM bank of fp32)
NJ = C // NF  # 2 output column halves

_FP32 = mybir.dt.float32
_BF16 = mybir.dt.bfloat16

G = 4          # row tiles covered by the k-outer fill phase
N_WARMUP = 12  # dummy matmuls to ramp the PE clock during the DMA fill
V_GROUPS = [(0, 2), (2, 4), (4, 8), (8, 12), (12, 16)]


def _build():
    nc = bacc.Bacc()
    vp = nc.dram_tensor("vp", [P, MT, KT, P], _BF16, kind="ExternalInput")
    wp = nc.dram_tensor("wp", [P, KT, C], _BF16, kind="ExternalInput")
    out = nc.dram_tensor("out", [T, C], _BF16, kind="ExternalOutput")

    with TileContext(nc) as tc:
        with (
            tc.tile_pool(name="wpool", bufs=1) as wpool,
            tc.tile_pool(name="vpool", bufs=1) as vpool,
            tc.tile_pool(name="spool", bufs=1) as spool,
            tc.tile_pool(name="opool", bufs=4) as opool,
            tc.tile_pool(name="pspool", bufs=4, space="PSUM") as pspool,
        ):
            # PE warmup: dependency-free matmuls on a memset tile so the HAM
            # clock gate ramps to 2.4 GHz while the first DMAs are in flight.
            scratch = spool.tile([P, NF], _BF16, name="scratch", tag="scratch")
            nc.gpsimd.memset(scratch, 0.0)
            ps_w = pspool.tile([P, NF], _FP32, name="ps_w", tag="ps")
            for _ in range(N_WARMUP):
                nc.tensor.matmul(
                    ps_w, lhsT=scratch[:, :P], rhs=scratch, start=True, stop=True
                )

            # Input DMAs.  w chunks (2 k-tiles x full C) on the scalar
            # engine's queue; v strip groups on the sync engine's queue in
            # parallel.  Both are HW-DGE paths (gpsimd would be SW-DGE and
            # adds DMASW teardown rounds at kernel exit).
            w_sb = []
            for c in range(KT // 2):
                w_c = wpool.tile([P, 2, C], _BF16, name=f"w_{c}", tag=f"w_{c}")
                nc.scalar.dma_start(out=w_c, in_=wp[:, 2 * c : 2 * c + 2, :])
                w_sb.append(w_c)
            v_sb = [None] * MT
            for lo, hi in V_GROUPS:
                vt = vpool.tile(
                    [P, hi - lo, KT, P], _BF16, name=f"v_{lo}", tag=f"v_{lo}"
                )
                nc.sync.dma_start(out=vt, in_=vp[:, lo:hi, :, :])
                for m in range(lo, hi):
                    v_sb[m] = (vt, m - lo)

            def mm(ps_m, m, k, j):
                vt, s = v_sb[m]
                nc.tensor.matmul(
                    ps_m[:, j * NF : (j + 1) * NF],
                    lhsT=vt[:, s, k, :],
                    rhs=w_sb[k // 2][:, k % 2, j * NF : (j + 1) * NF],
                    start=(k == 0),
                    stop=(k == KT - 1),
                )

            ps = {}

            def drain(m, ob_m, j=None):
                sl = slice(None) if j is None else slice(j * NF, (j + 1) * NF)
                nc.vector.tensor_copy(ob_m[:, sl], ps[m][:, sl])
                nc.sync.dma_start(out=out[m * P : (m + 1) * P, sl], in_=ob_m[:, sl])

            # Fill phase (m0-3): k-outer so each arriving w chunk immediately
            # feeds 16 matmuls -- the PE never idles while w streams in.
            for m in range(G):
                ps[m] = pspool.tile([P, C], _FP32, name=f"ps_{m}", tag="ps")
            for c in range(KT // 2):
                for m in range(G):
                    for kk in range(2):
                        for j in range(NJ):
                            mm(ps[m], m, 2 * c + kk, j)
                    if c == KT // 2 - 1:
                        ob_m = opool.tile([P, C], _BF16, name=f"ob_{m}", tag="ob")
                        drain(m, ob_m)

            # Steady phase (m4-15): m-major, one drain per row tile.  The
            # last tile runs j-outer and drains in halves so only half a
            # drain trails the final matmul.
            for m in range(G, MT):
                ps[m] = pspool.tile([P, C], _FP32, name=f"ps_{m}", tag="ps")
                ob_m = opool.tile([P, C], _BF16, name=f"ob_{m}", tag="ob")
                if m < MT - 1:
                    for k in range(KT):
                        for j in range(NJ):
                            mm(ps[m], m, k, j)
                    drain(m, ob_m)
                else:
                    for j in range(NJ):
                        for k in range(KT):
                            mm(ps[m], m, k, j)
                        drain(m, ob_m, j=j)
    nc.compile()
    return nc


_nc_cache = None


def _get_nc():
    global _nc_cache
    if _nc_cache is None:
        _nc_cache = _build()
    return _nc_cache


def prepare_inputs(inputs):
    """Host-side prep shared by kernel() and the timing harness.

    Returns (in_maps, b0): per-core device inputs and the bias to add on
    the host after the gather.
    """
    import ml_dtypes

    v = np.ascontiguousarray(np.asarray(inputs["v"], dtype=np.float32))
    Wv = np.asarray(inputs["Wv"], dtype=np.float32)
    W0 = np.asarray(inputs["W0"], dtype=np.float32)
    b0 = np.asarray(inputs["b0"], dtype=np.float32)

    # Fuse the two linear layers on the host: WcT = (W0 @ Wcat)^T  [C_in, C_out]
    Wc = W0 @ Wv.reshape(H * D, C)
    # wp[p, k, j] = WcT[k*128+p, j]
    wp = np.ascontiguousarray(
        Wc.T.reshape(KT, P, C).transpose(1, 0, 2).astype(ml_dtypes.bfloat16)
    )
    # vp[b][p, m, k, t] = v[b, m*128+t, k*128+p]
    vp = np.ascontiguousarray(
        v.reshape(B, MT, P, KT, P).transpose(0, 4, 1, 3, 2).astype(ml_dtypes.bfloat16)
    )
    return [{"vp": vp[i], "wp": wp} for i in range(B)], b0


def kernel(**inputs):
    in_maps, b0 = prepare_inputs(inputs)
    nc = _get_nc()
    res = run_bass_kernel_spmd(nc, in_maps, core_ids=list(range(B)))
    out = np.stack([res.results[i]["out"] for i in range(B)], axis=0)
    return out.astype(np.float32) + b0
